# revision 1
# baseline (speedup 1.0000x reference)
"""MoE low-rank adapters (top-1 routing) Trainium2 kernel.

Math (reference):
  xf = x.reshape(N, D)                 N=8192, D=2048, E=8, R=64
  logits = xf @ Wg.T                   [N, E]
  prob = softmax(logits); gate = argmax(prob); prob_sel = max(prob)
  h = xf @ A[e].T for all e            [N, E*R]
  y = (h * onehot(gate)) @ Bwt         [N, D]
  y *= SCALING * prob_sel

Distribution: data-parallel over tokens, 8 cores x 1024 tokens.

Per-core device pipeline (4 blocks of 256 tokens):
  - gating logits^T via col-tiled exact-fp32 matmuls (stationary Wg^T chunks
    [128,8] at 4 PE column groups; moving x^T fp32)
  - h^T = A_t.T @ x_hi in bf16 (A_t host-transposed bf16; x_hi on-chip cast)
  - softmax/argmax epilogue in natural layout after tiny PE transposes;
    mask*scale coefficient maskval[tok,e] = (logit==max) * SCALING/sumexp
  - maskval transposed (PE) -> staged to DRAM -> partition-broadcast DMA
    to expand over the 64 ranks of each expert -> h_masked^T (f32r)
  - y = h_masked^T.T @ Bwt in f32r (Bwt host-transposed, f32r-typed)
"""

import sys
import os

for _p in ("/opt/trn_rl_repo",):
    if _p not in sys.path:
        sys.path.insert(0, _p)

import numpy as np
import ml_dtypes

import concourse.bass as bass
import concourse.bacc as bacc
import concourse.mybir as mybir
import concourse.tile as tile
from concourse import bass_utils
from concourse.masks import make_identity

f32 = mybir.dt.float32
f32r = mybir.dt.float32r
bf16 = mybir.dt.bfloat16

B, S, D, R, E = 4, 2048, 2048, 64, 8
N = B * S                    # 8192 tokens
NCORES = 8
NTOK = N // NCORES           # 1024 tokens per core
SCALING = 64.0 / 16.0
ER = E * R                   # 512
KD = D // 128                # 16 d-chunks
BLK = 256                    # tokens per block
NBLK = NTOK // BLK           # 4 blocks
TCH = BLK // 128             # tok-chunks per block (2)
NOCH = 4                     # output chunks of 512
ERCH = ER // 128             # er chunks (4)

_CACHE = {}


def _build():
    if "nc" in _CACHE:
        return _CACHE["nc"]
    nc = bacc.Bacc("TRN2", target_bir_lowering=False, debug=False,
                   num_devices=NCORES)
    xt = nc.dram_tensor("xt", [D, NTOK], f32, kind="ExternalInput")
    abf = nc.dram_tensor("abf", [D, ER], bf16, kind="ExternalInput")
    bwr = nc.dram_tensor("bwr", [ER, D], f32r, kind="ExternalInput")
    wg = nc.dram_tensor("wg", [D, E], f32, kind="ExternalInput")
    sel = nc.dram_tensor("sel", [128, E], f32, kind="ExternalInput")
    yo = nc.dram_tensor("yo", [NTOK, D], f32, kind="ExternalOutput")
    mstage = nc.dram_tensor("mstage", [NBLK // 2, 4, E, 128], f32, kind="Internal")

    with tile.TileContext(nc) as tc:
        import contextlib
        ctx = contextlib.ExitStack()
        with ctx:
            singles = ctx.enter_context(tc.tile_pool(name="singles", bufs=1))
            xpool = ctx.enter_context(tc.tile_pool(name="xpool", bufs=3))
            hpool = ctx.enter_context(tc.tile_pool(name="hpool", bufs=2))
            mpool = ctx.enter_context(tc.tile_pool(name="mpool", bufs=2))
            spool = ctx.enter_context(tc.tile_pool(name="spool", bufs=3))
            ypool = ctx.enter_context(tc.tile_pool(name="ypool", bufs=3))
            ps_h = ctx.enter_context(tc.tile_pool(name="ps_h", bufs=1, space="PSUM"))
            ps_lg = ctx.enter_context(tc.tile_pool(name="ps_lg", bufs=1, space="PSUM"))
            ps_tr = ctx.enter_context(tc.tile_pool(name="ps_tr", bufs=1, space="PSUM"))
            ps_y = ctx.enter_context(tc.tile_pool(name="ps_y", bufs=1, space="PSUM"))

            # ---- weights (loaded once) ----
            # gating weights + selector first (block 0's x load follows on the
            # same sync queue); big adapter weights go on the scalar HWDGE
            # queue so they don't serialize in front of x.
            wg_sb = singles.tile([128, KD, E], f32)
            nc.sync.dma_start(out=wg_sb,
                              in_=wg.ap().rearrange("(k p) e -> p k e", p=128))
            sel_sb = singles.tile([128, E], f32)
            nc.sync.dma_start(out=sel_sb, in_=sel.ap())
            ident = singles.tile([128, 128], f32)
            make_identity(nc, ident)
            identb = singles.tile([128, 128], bf16)
            make_identity(nc, identb)

            # ---- x^T resident in SBUF, one DMA per d-chunk (4KB runs) ----
            # chunk order matches gating group-major consumption; chunks
            # alternate between the two HWDGE queues. Adapter weights queue
            # behind x on the scalar queue (h starts after gating, y later).
            KORDER = [4 * r + g for g in range(4) for r in range(4)]
            xchunk = []
            for k in range(KD):
                xck = singles.tile([128, NTOK], f32, tag=f"xc{k}")
                xchunk.append(xck)
            abf_sb = singles.tile([128, KD, ER], bf16)
            bwr_sb = singles.tile([128, ERCH, D], f32r)
            for idx, k in enumerate(KORDER):
                eng = nc.sync if idx % 2 == 0 else nc.gpsimd
                eng.dma_start(
                    out=xchunk[k],
                    in_=xt.ap()[128 * k:128 * k + 128, :])
            nc.gpsimd.dma_start(out=abf_sb,
                                in_=abf.ap().rearrange("(k p) e -> p k e", p=128))
            nc.gpsimd.dma_start(out=bwr_sb,
                                in_=bwr.ap().rearrange("(i p) o -> p i o", p=128))

            # y PSUM banks rotate over three slots (the third borrows the
            # gating bank) so each chain's bank-release copy is off the
            # critical path of the next chain
            yrot = [0]

            def ypsum():
                tag = ("y0", "y1", "lg")[yrot[0] % 3]
                yrot[0] += 1
                pool = ps_lg if tag == "lg" else ps_y
                t = pool.tile([128, 512], f32, tag=tag, name=f"yps_{yrot[0]}")
                return t

            def emit_y(n0, hmT):
                # y = hmT.T @ Bwt (f32r); stationary hmT_i reused for two
                # o-chunks per load so the weight path stays hidden
                for t in range(TCH):
                    ysb = ypool.tile([128, D], f32, tag="ysb")
                    for jp in range(NOCH // 2):
                        yp0 = ypsum()
                        yp1 = ypsum()
                        for i in range(ERCH):
                            hslice = hmT[i][:, 128 * t:128 * t + 128]
                            o0 = 1024 * jp
                            nc.tensor.matmul(
                                yp0, hslice, bwr_sb[:, i, o0:o0 + 512],
                                start=(i == 0), stop=(i == ERCH - 1))
                            nc.tensor.matmul(
                                yp1, hslice, bwr_sb[:, i, o0 + 512:o0 + 1024],
                                start=(i == 0), stop=(i == ERCH - 1))
                        o0c = 1024 * jp
                        nc.scalar.copy(ysb[:, o0c:o0c + 256], yp0[:, 0:256])
                        nc.vector.tensor_copy(ysb[:, o0c + 256:o0c + 512],
                                              yp0[:, 256:512])
                        nc.scalar.copy(ysb[:, o0c + 512:o0c + 768],
                                       yp1[:, 0:256])
                        nc.vector.tensor_copy(ysb[:, o0c + 768:o0c + 1024],
                                              yp1[:, 256:512])
                        nc.sync.dma_start(
                            out=yo.ap()[n0 + 128 * t:n0 + 128 * t + 128,
                                        o0c:o0c + 1024],
                            in_=ysb[:, o0c:o0c + 1024])

            PBLK = 2 * BLK       # 512-token gating pair

            # PE warm-up while the first x chunks stream in: ~4.5us of dummy
            # matmuls lifts the HAM clock gate to 2.4GHz before real work
            wup = ps_y.tile([128, 512], f32, tag="y0", name="warmup_ps")
            for w in range(55):
                nc.tensor.matmul(wup[:, 0:128], identb, identb,
                                 start=True, stop=True)

            def prep_lg():
                # fresh gating PSUM bank, zeroed: the col-tiled matmuls only
                # write 4x8 partition bands; the epilogue copy reads all 128
                # rows and uninitialized PSUM may hold NaN (NaN * 0 = NaN).
                t = ps_lg.tile([128, PBLK], f32, tag="lg", name="lg_ps")
                nc.vector.memset(t, 0.0)
                return t

            lg_next = prep_lg()
            pending_y = None
            for pair in range(NBLK // 2):
                p0 = pair * PBLK
                xp32 = [xchunk[k][:, p0:p0 + PBLK] for k in range(KD)]

                # ---- gating over the pair: col-tiled exact fp32 ----
                # memset first: the col-tiled matmuls only write 4x8 partition
                # bands; the epilogue copy reads all 128 rows and uninitialized
                # PSUM may hold NaN (NaN * SEL(=0) = NaN).
                lg_ps = lg_next
                for g in range(4):
                    for r in range(KD // 4):
                        k = 4 * r + g
                        nc.tensor.matmul(
                            lg_ps[32 * g:32 * g + E, :],
                            wg_sb[:, k, :],
                            xp32[k],
                            start=(r == 0), stop=(r == KD // 4 - 1),
                            tile_position=(0, 32 * g),
                        )


                # half-0 bf16 casts first: they gate the h matmuls and must
                # not queue behind the epilogue chain on ACT/DVE
                blk0 = 2 * pair
                n0_0 = blk0 * BLK
                xhi0 = xpool.tile([128, KD, BLK], bf16, tag="xhi")
                for idx, k in enumerate(KORDER):
                    if idx % 2 == 0:
                        nc.scalar.copy(xhi0[:, k, :],
                                       xchunk[k][:, n0_0:n0_0 + BLK])
                    else:
                        nc.vector.tensor_copy(xhi0[:, k, :],
                                              xchunk[k][:, n0_0:n0_0 + BLK])

                # ---- gating epilogue (pair-wide, stage-major) ----
                # trp (sel-matmul outputs, [128, 8] per tok-chunk) and the
                # fused maskval transpose share one PSUM bank.
                lg_sb = spool.tile([128, PBLK], f32, tag="lg_sb")
                eptr = ps_tr.tile([128, 512], f32, tag="tr")
                NT = PBLK // 128
                for t in range(NT):
                    nc.scalar.copy(lg_sb[:, 128 * t:128 * t + 128],
                                   lg_ps[:, 128 * t:128 * t + 128])
                for t in range(NT):
                    nc.tensor.matmul(eptr[:, 8 * t:8 * t + 8],
                                     lg_sb[:, 128 * t:128 * t + 128],
                                     sel_sb, start=True, stop=True)
                mxs, negs, ses = [], [], []
                for t in range(NT):
                    mx = spool.tile([128, 1], f32, tag=f"mx{t}")
                    nc.vector.reduce_max(out=mx, in_=eptr[:, 8 * t:8 * t + 8],
                                         axis=mybir.AxisListType.X)
                    mxs.append(mx)
                for t in range(NT):
                    negmx = spool.tile([128, 1], f32, tag=f"negmx{t}")
                    nc.vector.tensor_scalar_mul(negmx, mxs[t], -1.0)
                    negs.append(negmx)
                for t in range(NT):
                    es = spool.tile([128, 8], f32, tag=f"es{t}")
                    se = spool.tile([128, 1], f32, tag=f"se{t}")
                    nc.scalar.activation(out=es, in_=eptr[:, 8 * t:8 * t + 8],
                                         func=mybir.ActivationFunctionType.Exp,
                                         bias=negs[t], scale=1.0, accum_out=se)
                    ses.append(se)
                rcps = []
                for t in range(NT):
                    rcp = spool.tile([128, 1], f32, tag=f"rcp{t}")
                    nc.vector.reciprocal(rcp, ses[t])
                    rcps.append(rcp)
                # maskval[tok, e] = (logit == max) * prob_sel  (SCALING is
                # folded into Bwt host-side)
                mval4 = spool.tile([128, NT, 8], f32, tag="mval4")
                for t in range(NT):
                    nc.vector.tensor_scalar(
                        out=mval4[:, t, :], in0=eptr[:, 8 * t:8 * t + 8],
                        scalar1=mxs[t], scalar2=rcps[t],
                        op0=mybir.AluOpType.is_equal, op1=mybir.AluOpType.mult)
                # one fused transpose: [128 tok, (t,e)=32] -> [(t,e)=32, 128]
                nc.tensor.transpose(eptr[0:32, 128:256],
                                    mval4.rearrange("p t e -> p (t e)"), ident)
                mvT4 = mpool.tile([32, 128], f32, tag="mvT4")
                nc.scalar.copy(mvT4, eptr[0:32, 128:256])

                # stage to DRAM, broadcast over the 64 ranks of each expert
                nc.gpsimd.dma_start(out=mstage.ap()[pair], in_=mvT4)
                mexp = []
                for i in range(ERCH):
                    me = mpool.tile([128, PBLK], f32, tag=f"me{i}")
                    mexp.append(me)
                    for half_e in range(2):
                        srcap = bass.AP(
                            tensor=mstage,
                            offset=(pair * (NT * E * 128)
                                    + (2 * i + half_e) * 128),
                            ap=[[0, 64], [E * 128, NT], [1, 128]],
                        )
                        nc.gpsimd.dma_start(
                            out=me[64 * half_e:64 * half_e + 64, :]
                            .rearrange("p (t n) -> p t n", t=NT),
                            in_=srcap)

                # half 0: h + previous y (PE fills while the mask path runs)
                h_ps0 = ps_h.tile([128, ERCH, BLK], f32, tag="h")
                for i in range(ERCH):
                    for kk, k in enumerate(KORDER):
                        nc.tensor.matmul(
                            h_ps0[:, i, :],
                            abf_sb[:, k, 128 * i:128 * i + 128],
                            xhi0[:, k, :],
                            start=(kk == 0), stop=(kk == KD - 1))
                hmT0 = []
                for i in range(ERCH):
                    hm = hpool.tile([128, BLK], f32r, tag=f"hm{i}")
                    nc.vector.tensor_mul(hm, h_ps0[:, i, :],
                                         mexp[i][:, 0:BLK])
                    hmT0.append(hm)
                if pending_y is not None:
                    emit_y(*pending_y)
                    pending_y = None

                # half 1: casts + h, then half-0 y, then half-1 mask
                blk1 = 2 * pair + 1
                n0_1 = blk1 * BLK
                xhi1 = xpool.tile([128, KD, BLK], bf16, tag="xhi")
                for idx, k in enumerate(KORDER):
                    if idx % 2 == 0:
                        nc.scalar.copy(xhi1[:, k, :],
                                       xchunk[k][:, n0_1:n0_1 + BLK])
                    else:
                        nc.vector.tensor_copy(xhi1[:, k, :],
                                              xchunk[k][:, n0_1:n0_1 + BLK])
                h_ps1 = ps_h.tile([128, ERCH, BLK], f32, tag="h")
                for i in range(ERCH):
                    for kk, k in enumerate(KORDER):
                        nc.tensor.matmul(
                            h_ps1[:, i, :],
                            abf_sb[:, k, 128 * i:128 * i + 128],
                            xhi1[:, k, :],
                            start=(kk == 0), stop=(kk == KD - 1))
                hmT1 = []
                for i in range(ERCH):
                    hm = hpool.tile([128, BLK], f32r, tag=f"hm{i}")
                    nc.vector.tensor_mul(hm, h_ps1[:, i, :],
                                         mexp[i][:, BLK:PBLK])
                    hmT1.append(hm)
                emit_y(n0_0, hmT0)
                pending_y = (n0_1, hmT1)

            emit_y(*pending_y)

    nc.compile()
    _CACHE["nc"] = nc
    return nc


def _prep_inputs(x, A, Bw, Wg):
    xf = np.ascontiguousarray(np.asarray(x, dtype=np.float32).reshape(N, D))
    xT = np.ascontiguousarray(xf.T)                              # [D, N]
    A_t = np.ascontiguousarray(
        np.asarray(A, dtype=np.float32).reshape(ER, D).T).astype(ml_dtypes.bfloat16)
    Bwt = np.ascontiguousarray(
        np.asarray(Bw, dtype=np.float32).transpose(0, 2, 1).reshape(ER, D)
        * SCALING)
    WgT = np.ascontiguousarray(np.asarray(Wg, dtype=np.float32).T)  # [D, E]
    SEL = np.zeros((128, E), dtype=np.float32)
    for p in range(128):
        if p % 32 < E:
            SEL[p, p % 32] = 1.0
    in_maps = []
    for c in range(NCORES):
        in_maps.append({
            "xt": np.ascontiguousarray(xT[:, c * NTOK:(c + 1) * NTOK]),
            "abf": A_t,
            "bwr": Bwt,
            "wg": WgT,
            "sel": SEL,
        })
    return in_maps


def _run(x, A, Bw, Wg, trace=False):
    nc = _build()
    in_maps = _prep_inputs(x, A, Bw, Wg)
    res = bass_utils.run_bass_kernel_spmd(
        nc, in_maps, core_ids=list(range(NCORES)), trace=trace)
    y = np.concatenate([res.results[c]["yo"] for c in range(NCORES)], axis=0)
    return y.reshape(B, S, D), res


def kernel(x, A, Bw, Wg):
    y, _ = _run(x, A, Bw, Wg, trace=False)
    return y

